# revision 1
# baseline (speedup 1.0000x reference)
"""Trainium2 Bass kernel for nn_Encoder_62740882260638 (ragged set encoder).

Pure data parallel over 8 NeuronCores: each core handles B/8 = 1024 of the
8192 sets; weights replicated; no collectives.

Key restructurings vs. the jax reference:
  * No physical sort: each element's stable-argsort rank is computed by
    pairwise comparison counting, and position-dependent quantities are
    gathered BY RANK with one-hot matmuls on the tensor engine.  Padded
    elements get rank 16 and are routed to "trash" table rows (0 for the
    deepset key table => masks the deepset sum; -1e30 for the km first-layer
    rows => relu zeroes the hidden => masks the main sum), so the ragged
    masking costs nothing.
  * The kd position-key MLP depends only on position => host-precomputed
    17x256 table.  All biases are folded into tables / extra contraction
    rows / fused ops.
  * Activations are feature-major [feat(part) x tokens(free)] so matmuls
    chain without transposes; x is PE-transposed on entry, z on exit.
  * Matmuls run float32r (1 col/cyc @ N=512); the rank-score matmul runs
    true float32 so the permutation matches the fp32 reference.
"""

from contextlib import ExitStack

import numpy as np

import concourse.bass as bass
import concourse.mybir as mybir
import concourse.tile as tile
from concourse import bacc
from concourse import bass_utils

FP = mybir.dt.float32
FPR = mybir.dt.float32r
I32 = mybir.dt.int32
AF = mybir.ActivationFunctionType
OP = mybir.AluOpType

B, N, DIM, HID, MAXN1 = 8192, 16, 256, 512, 17
NCORES = 8
SC = B // NCORES  # sets per core (1024)
CS = 128          # sets per chunk
CT = CS * N       # tokens per chunk (2048)
NSUB = 512        # tokens per sub-chunk (matmul N)
BIG = 1.0e30


def _ksplit(total):
    return [(a, min(a + 128, total)) for a in range(0, total, 128)]


def build_program(sets_per_core=SC, num_devices=1, debug_taps=False):
    nc = bacc.Bacc(
        "TRN2", target_bir_lowering=False, debug=False,
        num_devices=num_devices,
    )
    S = sets_per_core
    assert S % CS == 0
    nchunks = S // CS
    T = S * N

    def din(name, shape, dtype=FP):
        return nc.dram_tensor(name, shape, dtype, kind="ExternalInput").ap()

    x_d = din("x", [T, DIM])
    n_d = din("n_i", [S], I32)

    vdW1 = din("vdW1", [DIM, DIM], FPR)
    b1d = din("b1d", [DIM, 1])
    vdW2 = din("vdW2", [DIM, DIM], FPR)
    b2d = din("b2d", [DIM, 1])
    vmW1x = din("vmW1x", [DIM, HID], FPR)
    b1v = din("b1v", [HID, 1])
    vmW2 = din("vmW2", [HID, HID], FPR)
    b2v = din("b2v", [HID, 1])
    Kds = din("Kds", [128, DIM], FPR)
    KRt = din("KRt", [128, 392], FPR)
    kmW2e = din("kmW2e", [393, HID], FPR)
    edW1 = din("edW1", [DIM, DIM], FPR)
    edb1 = din("edb1", [DIM, 1])
    edW2 = din("edW2", [DIM, DIM], FPR)
    Wze = din("Wze", [DIM + 1, 904], FPR)
    emW1y = din("emW1y", [HID, 520], FPR)
    EMt = din("EMt", [MAXN1, 520], FPR)
    emW2e = din("emW2e", [521, HID], FPR)
    rankWb = din("rankWb", [128, DIM])
    ident = din("ident", [128, 128], FPR)
    I32r = din("I32r", [128, NSUB], FPR)
    iota113 = din("iota113", [128, 1])
    iota17 = din("iota17", [MAXN1, 1])
    S8 = din("S8", [128, 8], FPR)
    iota8_0 = din("iota8_0", [8, 1])
    iota8_1 = din("iota8_1", [8, 1])
    LT0 = din("LT0", [128, 1])
    LT1 = din("LT1", [128, 1])
    onesr = din("onesr", [1, CS], FPR)
    identF = din("identF", [128, 128])

    z_d = nc.dram_tensor("z_out", [S, HID], FP, kind="ExternalOutput").ap()
    magt_d = rankt_d = None
    if debug_taps:
        magt_d = nc.dram_tensor(
            "mag_tap", [CT], FP, kind="ExternalOutput").ap()
        rankt_d = nc.dram_tensor(
            "rank_tap", [CT], FP, kind="ExternalOutput").ap()

    with tile.TileContext(nc) as tc, ExitStack() as ctx, \
            nc.allow_low_precision(reason="f32r stores fp32 bits"):
        wpool = ctx.enter_context(tc.tile_pool(name="wpool", bufs=1))
        glob = ctx.enter_context(tc.tile_pool(name="glob", bufs=2))
        work = ctx.enter_context(tc.tile_pool(name="work", bufs=2))
        work3 = ctx.enter_context(tc.tile_pool(name="work3", bufs=2))
        ps = ctx.enter_context(tc.tile_pool(name="ps", bufs=1, space="PSUM"))
        dstg = ctx.enter_context(
            tc.tile_pool(name="dstg", bufs=2, space="DRAM"))

        def wload(ap, name):
            """Load a [K, M] weight as a list of <=128-partition K-slabs."""
            k = ap.shape[0]
            if len(ap.shape) == 1 or k <= 128:
                t = wpool.tile(ap.shape, ap.dtype, name=f"w_{name}")
                nc.sync.dma_start(out=t, in_=ap)
                return t
            slabs = []
            for si, (k0, k1) in enumerate(_ksplit(k)):
                t = wpool.tile([k1 - k0] + list(ap.shape[1:]), ap.dtype,
                               name=f"w_{name}_{si}")
                nc.sync.dma_start(out=t, in_=ap[k0:k1])
                slabs.append(t)
            return slabs

        s_vdW1 = wload(vdW1, "vdW1")
        s_b1d = wload(b1d, "b1d")
        s_vdW2 = wload(vdW2, "vdW2")
        s_b2d = wload(b2d, "b2d")
        s_vmW1x = wload(vmW1x, "vmW1x")
        s_b1v = wload(b1v, "b1v")
        s_vmW2 = wload(vmW2, "vmW2")
        s_b2v = wload(b2v, "b2v")
        s_Kds = wload(Kds, "Kds")
        s_KRt = wload(KRt, "KRt")
        s_kmW2e = wload(kmW2e, "kmW2e")
        s_edW1 = wload(edW1, "edW1")
        s_edb1 = wload(edb1, "edb1")
        s_edW2 = wload(edW2, "edW2")
        s_Wze = wload(Wze, "Wze")
        s_emW1y = wload(emW1y, "emW1y")
        s_EMt = wload(EMt, "EMt")
        s_emW2e = wload(emW2e, "emW2e")
        s_rankWb = wload(rankWb, "rankWb")
        s_ident = wload(ident, "ident")
        s_I32r = wload(I32r, "I32r")
        s_iota113 = wload(iota113, "iota113")
        s_iota17 = wload(iota17, "iota17")
        s_S8 = wload(S8, "S8")
        s_iota8_0 = wload(iota8_0, "iota8_0")
        s_iota8_1 = wload(iota8_1, "iota8_1")
        s_LT0 = wload(LT0, "LT0")
        s_LT1 = wload(LT1, "LT1")
        s_ones = wload(onesr, "onesr")
        s_identF = wload(identF, "identF")

        def psum(name, shape=(128, NSUB), tag="mmA", bufs=3, dtype=FP):
            return ps.tile(list(shape), dtype, name=name, tag=tag, bufs=bufs)

        def acopy(out, in_):
            nc.scalar.activation(out, in_, AF.Copy)

        def mm_acc(pt, slabs, msl, rhs_parts, extra=None, dtype=FPR,
                   keep_open=False):
            """pt += W[:, msl].T @ rhs for one <=128-wide M slice `msl`.

            slabs: list of [k_i, M] SBUF weight slabs covering K;
            rhs_parts: matching list of [k_i, n] APs.
            extra: optional (lhsT2, rhs2) accumulated at the end.
            """
            if not isinstance(slabs, list):
                slabs = [slabs]
            assert len(slabs) == len(rhs_parts)
            nk = len(rhs_parts) + (1 if extra is not None else 0) + 10**6 * 0
            if keep_open:
                nk = len(rhs_parts) + 1
            for i, (sl, rp) in enumerate(zip(slabs, rhs_parts)):
                assert sl.shape[0] == rp.shape[0], (sl.shape, rp.shape)
                nc.tensor.matmul(
                    pt, sl[:, msl].bitcast(dtype), rp.bitcast(dtype),
                    start=(i == 0), stop=(i == nk - 1))
            if extra is not None:
                l2, r2 = extra
                nc.tensor.matmul(
                    pt, l2.bitcast(dtype), r2.bitcast(dtype),
                    start=False, stop=True)

        x_r = x_d.rearrange("(c a p) d -> c a p d", c=nchunks, p=128)

        for c in range(nchunks):
            s0 = c * CS
            # ===== phase 1a: load x, transpose to feature-major, rank score
            x_fm = [work.tile([128, CT], FPR, name=f"x_fm{h}")
                    for h in (0, 1)]
            magd = dstg.tile([CT], FP, name="magd")
            for ns in range(4):
                tsl = slice(NSUB * ns, NSUB * (ns + 1))
                x_tm = work3.tile([128, 4, DIM], FP, name="x_tm")
                nc.sync.dma_start(
                    out=x_tm,
                    in_=x_r[c, 4 * ns:4 * ns + 4].transpose([1, 0, 2]))
                for h in range(2):
                    pt = psum("pt", (128, 4, 128), tag="tr", bufs=2, dtype=FP)
                    for a in range(4):
                        nc.tensor.transpose(
                            pt[:, a, :],
                            x_tm[:, a, 128 * h:128 * (h + 1)],
                            s_identF)
                    dst = x_fm[h][:, tsl].rearrange("p (a q) -> p a q", a=4)
                    if h == 0:
                        acopy(dst, pt)
                    else:
                        nc.vector.tensor_copy(out=dst, in_=pt)
                # rank scores via gpsimd (full fp32 bits -> exact sort)
                mag4 = work3.tile([128, 4], FP, name="mag4")
                junk = work3.tile([128, DIM], FP, name="junk")
                for a in range(4):
                    nc.vector.scalar_tensor_tensor(
                        out=junk, in0=x_tm[:, a, :], scalar=0.0,
                        in1=s_rankWb, op0=OP.bypass, op1=OP.mult,
                        accum_out=mag4[:, a:a + 1])
                nc.sync.dma_start(
                    out=magd[NSUB * ns:NSUB * (ns + 1)]
                    .rearrange("(a p) -> p a", p=128), in_=mag4)

            # ===== phase 1b: ranks + one-hots =====
            if debug_taps and c == 0:
                nc.sync.dma_start(out=magt_d, in_=magd)
            mag_fm = glob.tile([N, CS], FP, name="mag_fm", bufs=1)
            nc.sync.dma_start(
                out=mag_fm, in_=magd.rearrange("(s i) -> i s", i=N))
            n_i32 = glob.tile([MAXN1, CS], I32, name="n_i32", bufs=1)
            nc.sync.dma_start(
                out=n_i32,
                in_=n_d[s0:s0 + CS].unsqueeze(0).broadcast_to([MAXN1, CS]))
            n_repf = glob.tile([MAXN1, CS], FP, name="n_repf", bufs=1)
            nc.vector.tensor_copy(out=n_repf, in_=n_i32)
            inv = glob.tile([N, CS], FP, name="inv", bufs=1)
            nc.vector.tensor_scalar(
                out=inv, in0=n_repf[0:N], scalar1=s_iota17[0:N], scalar2=None,
                op0=OP.is_le)
            mag_m = glob.tile([N, CS], FP, name="mag_m", bufs=1)
            nc.vector.scalar_tensor_tensor(
                out=mag_m, in0=inv, scalar=BIG, in1=mag_fm,
                op0=OP.mult, op1=OP.add)
            magd2 = dstg.tile([N * CS], FP, name="magd2")
            nc.sync.dma_start(out=magd2, in_=mag_m)
            magd2v = magd2.rearrange("(i s) -> i s", s=CS)
            X2 = glob.tile([128, CS], FP, name="X2", bufs=1)
            nc.sync.dma_start(
                out=X2, in_=magd2v.unsqueeze(0).broadcast_to([8, N, CS]))
            rankd = dstg.tile([CT], FPR, name="rankd")
            rankdT = rankd.rearrange("(s i) -> i s", i=N)
            for h in range(2):
                X1 = glob.tile([128, CS], FP, name="X1", bufs=1)
                nc.sync.dma_start(
                    out=X1,
                    in_=magd2v[8 * h:8 * h + 8, :].unsqueeze(1)
                    .broadcast_to([8, N, CS]))
                cmp = glob.tile([128, CS], FPR, name="cmp", bufs=1)
                eq = glob.tile([128, CS], FP, name="eq", bufs=1)
                nc.vector.tensor_tensor(
                    out=cmp, in0=X2, in1=X1, op=OP.is_lt)
                nc.vector.tensor_tensor(
                    out=eq, in0=X2, in1=X1, op=OP.is_equal)
                nc.vector.scalar_tensor_tensor(
                    out=cmp, in0=eq, scalar=(s_LT0 if h == 0 else s_LT1),
                    in1=cmp, op0=OP.mult, op1=OP.add)
                pr = psum("pr", (8, CS), tag="sm", bufs=1)
                nc.tensor.matmul(pr, s_S8, cmp)
                rh = glob.tile([8, CS], FP, name=f"rh{h}", bufs=1)
                nc.vector.tensor_copy(out=rh, in_=pr)
                # rank_m = rank + inv * (16 - rank), per 8-row half
                ih = glob.tile([8, CS], FP, name=f"ih{h}", bufs=1)
                nc.vector.tensor_scalar(
                    out=ih, in0=n_repf[0:8],
                    scalar1=(s_iota8_0 if h == 0 else s_iota8_1),
                    scalar2=None, op0=OP.is_le)
                th = glob.tile([8, CS], FPR, name=f"th{h}", bufs=1)
                nc.vector.tensor_scalar(
                    out=th, in0=rh, scalar1=-1.0, scalar2=16.0,
                    op0=OP.mult, op1=OP.add)
                nc.vector.tensor_tensor(
                    out=th, in0=th.bitcast(FP), in1=ih, op=OP.mult)
                nc.vector.tensor_tensor(
                    out=th, in0=th.bitcast(FP), in1=rh, op=OP.add)
                nc.sync.dma_start(out=rankdT[8 * h:8 * h + 8, :], in_=th)
            # one-hot tile: OH[32g + r, tok_of_subchunk_g] = (rank == r)
            OH = work.tile([128, NSUB], FPR, name="OH")
            for g in range(4):
                nc.sync.dma_start(
                    out=OH[32 * g:32 * (g + 1), :],
                    in_=rankd[NSUB * g:NSUB * (g + 1)].unsqueeze(0)
                    .broadcast_to([32, NSUB]))
            nc.vector.tensor_scalar(
                out=OH, in0=OH.bitcast(FP), scalar1=s_iota113, scalar2=None,
                op0=OP.is_equal)
            if debug_taps and c == 0:
                nc.sync.dma_start(out=rankt_d.bitcast(FPR), in_=rankd)
            # maskrow4[g, tok] = (rank_m < 16) = valid(token)
            maskrow4 = glob.tile([4, NSUB], FPR, name="maskrow4")
            nc.sync.dma_start(
                out=maskrow4, in_=rankd.rearrange("(g t) -> g t", g=4))
            nc.vector.tensor_scalar(
                out=maskrow4, in0=maskrow4.bitcast(FP), scalar1=15.5,
                scalar2=None, op0=OP.is_le)

            # ===== phase 1c: deepset branch =====
            y2ds = [glob.tile([128, CS], FPR, name=f"y2ds{m}")
                    for m in (0, 1)]
            for ns in range(4):
                tsl = slice(NSUB * ns, NSUB * (ns + 1))
                ssl = slice(32 * ns, 32 * (ns + 1))
                xp = [x_fm[0][:, tsl], x_fm[1][:, tsl]]
                oh = OH[32 * ns:32 * ns + MAXN1, :]
                Hd = []
                for m in range(2):
                    pd = psum(f"pd{m}")
                    mm_acc(pd, s_vdW1, slice(128 * m, 128 * (m + 1)), xp)
                    hd = work3.tile([128, NSUB], FPR, name=f"Hd{m}")
                    nc.scalar.activation(
                        hd, pd, AF.Relu, bias=s_b1d[m])
                    Hd.append(hd)
                for m in range(2):
                    pg = psum(f"pg{m}", tag="mmB", bufs=2)
                    nc.tensor.matmul(
                        pg,
                        s_Kds[32 * ns:32 * ns + MAXN1,
                              128 * m:128 * (m + 1)],
                        oh, tile_position=(32 * ns, 0))
                    kg = work3.tile([128, NSUB], FP, name="KG", tag="KG")
                    nc.vector.tensor_copy(out=kg, in_=pg)
                    pv = psum(f"pv{m}")
                    mm_acc(pv, s_vdW2, slice(128 * m, 128 * (m + 1)), Hd)
                    pds = psum(f"Pds{m}", tag="mmB", bufs=2)
                    nc.vector.scalar_tensor_tensor(
                        out=pds, in0=pv, scalar=s_b2d[m],
                        in1=kg, op0=OP.add, op1=OP.mult)
                    nc.vector.tensor_reduce(
                        out=y2ds[m][:, ssl],
                        in_=pds.rearrange("p (s i) -> p s i", i=N),
                        axis=mybir.AxisListType.X, op=OP.add)

            # ===== phase 2: ed MLP + z projections =====
            He = []
            for m in range(2):
                pe = psum(f"pe{m}", (128, CS))
                mm_acc(pe, s_edW1, slice(128 * m, 128 * (m + 1)), y2ds)
                he = glob.tile([128, CS], FPR, name=f"He{m}")
                nc.scalar.activation(
                    he, pe, AF.Relu, bias=s_edb1[m])
                He.append(he)
            ze = []
            for m in range(2):
                pz = psum(f"pz{m}", (128, CS), tag="mmB", bufs=2)
                mm_acc(pz, s_edW2, slice(128 * m, 128 * (m + 1)), He)
                z1 = glob.tile([128, CS], FPR, name=f"ze{m}")
                acopy(z1, pz)
                ze.append(z1)
            ones_r = s_ones
            zpT_s = glob.tile([128, 904], FPR, name="zpT_s")
            for half in range(2):
                csl = slice(452 * half, 452 * (half + 1))
                pzt = psum("pzt", (128, 452), tag="mmB", bufs=2)
                mm_acc(pzt, [ze[0], ze[1], ones_r], slice(None),
                       [sw[:, csl] for sw in s_Wze])
                acopy(zpT_s[:, csl], pzt)

            # ===== phase 3: main branch =====
            y2m = [glob.tile([128, CS], FPR, name=f"y2m{m}")
                   for m in range(4)]
            for ns in range(4):
                tsl = slice(NSUB * ns, NSUB * (ns + 1))
                ssl = slice(32 * ns, 32 * (ns + 1))
                xp = [x_fm[0][:, tsl], x_fm[1][:, tsl]]
                oh = OH[32 * ns:32 * ns + MAXN1, :]
                i32 = s_I32r[32 * ns:32 * (ns + 1), :]
                zsl = slice(32 * ns, 32 * (ns + 1))
                Hv = []
                for m in range(4):
                    pvm = psum(f"pvm{m}")
                    mm_acc(pvm, s_vmW1x, slice(128 * m, 128 * (m + 1)),
                           xp, keep_open=True)
                    nc.tensor.matmul(
                        pvm, zpT_s[zsl, 128 * m:128 * (m + 1)],
                        i32, start=False, stop=True,
                        tile_position=(32 * ns, 0))
                    hv = work3.tile([128, NSUB], FPR, name=f"Hv{m}")
                    nc.scalar.activation(
                        hv, pvm, AF.Relu, bias=s_b1v[m])
                    Hv.append(hv)
                Hk = []
                for m in range(4):
                    mw = 128 if m < 3 else 8
                    pkm = psum(f"pkm{m}")
                    nc.tensor.matmul(
                        pkm[:mw, :] if mw != 128 else pkm,
                        s_KRt[32 * ns:32 * ns + MAXN1,
                              128 * m:128 * m + mw],
                        oh, start=True, stop=False,
                        tile_position=(32 * ns, 0))
                    nc.tensor.matmul(
                        pkm[:mw, :] if mw != 128 else pkm,
                        zpT_s[zsl, 512 + 128 * m:512 + 128 * m + mw],
                        i32, start=False, stop=True,
                        tile_position=(32 * ns, 0))
                    hk = work3.tile(
                        [mw + 1 if m == 3 else mw, NSUB], FPR, name=f"Hk{m}")
                    nc.scalar.activation(
                        hk[:mw, :] if m == 3 else hk,
                        pkm[:mw, :] if mw != 128 else pkm, AF.Relu)
                    if m == 3:
                        nc.sync.dma_start(
                            out=hk[mw:mw + 1, :],
                            in_=maskrow4[ns:ns + 1, :])
                    Hk.append(hk)
                for m in range(4):
                    pK = psum(f"pK{m}", tag="mmB", bufs=2)
                    mm_acc(pK, s_kmW2e, slice(128 * m, 128 * (m + 1)), Hk)
                    km = work3.tile([128, NSUB], FP, name="Km", tag="Km")
                    acopy(km, pK)
                    pV = psum(f"pV{m}")
                    mm_acc(pV, s_vmW2, slice(128 * m, 128 * (m + 1)), Hv)
                    pmt = psum(f"Pm{m}", tag="mmB", bufs=2)
                    nc.vector.scalar_tensor_tensor(
                        out=pmt, in0=pV, scalar=s_b2v[m],
                        in1=km, op0=OP.add, op1=OP.mult)
                    nc.vector.tensor_reduce(
                        out=y2m[m][:, ssl],
                        in_=pmt.rearrange("p (s i) -> p s i", i=N),
                        axis=mybir.AxisListType.X, op=OP.add)

            # ===== phase 4: em MLP + output =====
            OHn = glob.tile([MAXN1, CS], FPR, name="OHn")
            nc.vector.tensor_scalar(
                out=OHn, in0=n_repf, scalar1=s_iota17, scalar2=None,
                op0=OP.is_equal)
            Hm = []
            for m in range(5):
                mw = 128 if m < 4 else 8
                pem = psum(f"pem{m}", (128, CS))
                pem_v = pem[:mw, :] if mw != 128 else pem
                mm_acc(pem_v, s_emW1y, slice(128 * m, 128 * m + mw), y2m,
                       extra=(s_EMt[:, 128 * m:128 * m + mw], OHn))
                hm = glob.tile(
                    [mw + 1 if m == 4 else mw, CS], FPR, name=f"Hm{m}")
                nc.scalar.activation(
                    hm[:mw, :] if m == 4 else hm, pem_v, AF.Relu)
                if m == 4:
                    nc.sync.dma_start(out=hm[mw:mw + 1, :], in_=ones_r)
                Hm.append(hm)
            zt = psum("zt", (128, 4, 128), tag="tr", bufs=2, dtype=FPR)
            for m in range(4):
                pzo = psum(f"pzo{m}", (128, CS), tag="mmB", bufs=2)
                mm_acc(pzo, s_emW2e, slice(128 * m, 128 * (m + 1)), Hm)
                zo = glob.tile([128, CS], FPR, name=f"zo{m}")
                acopy(zo, pzo)
                nc.tensor.transpose(zt[:, m, :], zo, s_ident)
            z_tm = glob.tile([128, 4, 128], FP, name="z_tm")
            nc.vector.tensor_copy(out=z_tm, in_=zt.bitcast(FP))
            nc.sync.dma_start(
                out=z_d[s0:s0 + CS, :].rearrange("s (m f) -> s m f", m=4),
                in_=z_tm)

    nc.compile()
    return nc


def make_tables(inp):
    """Host-side weight preprocessing -> dict of extra input arrays."""
    f = np.float32
    keys = ("rank_W", "kd_W1", "kd_b1", "kd_W2", "kd_b2", "vd_W1", "vd_b1",
            "vd_W2", "vd_b2", "ed_W1", "ed_b1", "ed_W2", "ed_b2", "km_W1",
            "km_b1", "km_W2", "km_b2", "vm_W1", "vm_b1", "vm_W2", "vm_b2",
            "em_W1", "em_b1", "em_W2", "em_b2")
    g = {k: np.asarray(inp[k], f) for k in keys}

    def A(v):
        return np.ascontiguousarray(v, dtype=f)

    kd_h = np.maximum(g["kd_W1"][:16] + g["kd_b1"][None, :], 0.0)
    Kds16 = kd_h @ g["kd_W2"] + g["kd_b2"][None, :]
    Kds17 = np.vstack([Kds16, np.zeros((1, DIM), f)])
    KRt17 = np.vstack([g["km_W1"][:16] + g["km_b1"][None, :],
                       np.full((1, 392), -BIG, f)])

    def rep4(tab):
        out = np.zeros((128, tab.shape[1]), f)
        for gi in range(4):
            out[32 * gi:32 * gi + MAXN1] = tab
        return out

    Kds = rep4(Kds17)
    KRt = rep4(KRt17)
    kmW2e = np.vstack([g["km_W2"], g["km_b2"][None, :]])

    Wz = np.hstack([g["vm_W1"][DIM:2 * DIM], g["km_W1"][MAXN1:MAXN1 + DIM]])
    Wze = np.vstack([Wz, (g["ed_b2"] @ Wz)[None, :]])

    EMt = g["em_W1"][HID:HID + MAXN1] + g["em_b1"][None, :]
    emW2e = np.vstack([g["em_W2"], g["em_b2"][None, :]])

    p = np.arange(128)
    iota113 = np.where(p % 32 < MAXN1, p % 32, 99).astype(f)[:, None]
    t = np.arange(NSUB)
    i32g = (t[None, :] // N == np.arange(32)[:, None]).astype(f)
    I32r = np.zeros((128, NSUB), f)
    for gi in range(4):
        I32r[32 * gi:32 * (gi + 1)] = i32g
    S8 = (p[:, None] // 16 == np.arange(8)[None, :]).astype(f)
    LT0 = ((p % 16) < (p // 16)).astype(f)[:, None]
    LT1 = ((p % 16) < (p // 16 + 8)).astype(f)[:, None]

    return {
        "vdW1": A(g["vd_W1"]), "b1d": A(g["vd_b1"][:, None]),
        "vdW2": A(g["vd_W2"]), "b2d": A(g["vd_b2"][:, None]),
        "vmW1x": A(g["vm_W1"][:DIM]), "b1v": A(g["vm_b1"][:, None]),
        "vmW2": A(g["vm_W2"]), "b2v": A(g["vm_b2"][:, None]),
        "Kds": A(Kds), "KRt": A(KRt), "kmW2e": A(kmW2e),
        "edW1": A(g["ed_W1"]), "edb1": A(g["ed_b1"][:, None]),
        "edW2": A(g["ed_W2"]), "Wze": A(Wze),
        "emW1y": A(g["em_W1"][:HID]), "EMt": A(EMt), "emW2e": A(emW2e),
        "rankWb": A(np.tile(g["rank_W"].T, (128, 1))),
        "ident": A(np.eye(128)), "I32r": A(I32r),
        "iota113": A(iota113),
        "iota17": A(np.arange(MAXN1)[:, None]),
        "S8": A(S8), "LT0": A(LT0), "LT1": A(LT1),
        "onesr": A(np.ones((1, CS))),
        "identF": A(np.eye(128)),
        "iota8_0": A(np.arange(8)[:, None]),
        "iota8_1": A(np.arange(8, 16)[:, None]),
    }


_prog_cache = {}


def _get_program(sets_per_core, num_devices):
    key = (sets_per_core, num_devices)
    if key not in _prog_cache:
        _prog_cache[key] = build_program(sets_per_core, num_devices)
    return _prog_cache[key]


def kernel(**inputs):
    nc = _get_program(SC, NCORES)
    tabs = make_tables(inputs)
    x = np.ascontiguousarray(np.asarray(inputs["x"], np.float32))
    n = np.ascontiguousarray(np.asarray(inputs["n"], np.int32))
    in_maps = []
    for c in range(NCORES):
        m = dict(tabs)
        m["x"] = np.ascontiguousarray(
            x[c * SC:(c + 1) * SC].reshape(SC * N, DIM))
        m["n_i"] = n[c * SC:(c + 1) * SC]
        in_maps.append(m)
    res = bass_utils.run_bass_kernel_spmd(nc, in_maps, list(range(NCORES)))
    z = np.concatenate([res.results[c]["z_out"] for c in range(NCORES)], 0)
    return z



# revision 26
# speedup vs baseline: 1.2544x; 1.2544x over previous
"""Trainium2 Bass kernel for nn_Encoder_62740882260638 (ragged set encoder).

Pure data parallel over 8 NeuronCores: each core handles B/8 = 1024 of the
8192 sets; weights replicated; no collectives.

Structure (v2 — software-pipelined two-pass):
  * Pass A (rank pipeline, issued 3 chunks ahead of pass B): loads x
    token-major, computes rank scores on DVE (exact fp32 accumulate ->
    matches the jax sort), masks padded slots, runs the comparison-count
    rank via small PE matmuls, builds the rank one-hot OH and the n one-hot
    OHn, and PE-transposes x to feature-major, staged to DRAM.
  * Pass B (per chunk): deepset branch, ed MLP (chunk-PAIRED so matmuls run
    256 cols — fp32r needs >=256 cols for 1 cycle/row), main branch, em MLP
    (also paired).  Issue order is software-pipelined:
    deepset(c) | rankA(c+3) | edpair | main(c-1) | rankB(c+3) | empair
    so the in-order PE queue never waits on the rank pipeline's DMA
    roundtrips and stays continuously busy (p-state ramp to 2.4 GHz).
  * Ragged masking is free: padded tokens get rank 16 -> routed to trash
    table rows (0 for the deepset key table; -1e30 for the km rows so relu
    zeroes the hidden).  The km mask row and em ones row are generated by
    EXTRA TABLE COLUMNS (a one-hot rhs sums to 1), not DMAs.
  * Engine split: PE matmuls; Act relus + psum copies + bulk DMA queue;
    DVE rank ops + product STTs; GpSimd segmented reduces + psum copies.
"""

from contextlib import ExitStack

import numpy as np

import concourse.bass as bass
import concourse.mybir as mybir
import concourse.tile as tile
from concourse import bacc
from concourse import bass_utils

FP = mybir.dt.float32
FPR = mybir.dt.float32r
I32 = mybir.dt.int32
AF = mybir.ActivationFunctionType
OP = mybir.AluOpType

B, N, DIM, HID, MAXN1 = 8192, 16, 256, 512, 17
NCORES = 8
SC = B // NCORES  # sets per core (1024)
CS = 128          # sets per chunk
CT = CS * N       # tokens per chunk (2048)
NSUB = 512        # tokens per sub-chunk (matmul N)
BIG = 1.0e30


def _ksplit(total):
    return [(a, min(a + 128, total)) for a in range(0, total, 128)]


def build_program(sets_per_core=SC, num_devices=1):
    nc = bacc.Bacc(
        "TRN2", target_bir_lowering=False, debug=False,
        num_devices=num_devices,
    )
    S = sets_per_core
    assert S % (2 * CS) == 0
    nchunks = S // CS

    def din(name, shape, dtype=FP):
        return nc.dram_tensor(name, shape, dtype, kind="ExternalInput").ap()

    x_d = din("x", [S * N, DIM])
    n_d = din("n_i", [S], I32)

    vdW1 = din("vdW1", [DIM, DIM], FPR)
    b1d = din("b1d", [DIM, 1])
    vdW2 = din("vdW2", [DIM, DIM], FPR)
    b2d = din("b2d", [DIM, 1])
    vmW1x = din("vmW1x", [DIM, HID], FPR)
    b1v = din("b1v", [HID, 1])
    vmW2 = din("vmW2", [HID, HID], FPR)
    b2v = din("b2v", [HID, 1])
    Kds = din("Kds", [128, DIM], FPR)
    KRt = din("KRt", [128, 394], FPR)
    kmW2e = din("kmW2e", [394, HID], FPR)
    edW1 = din("edW1", [DIM, DIM], FPR)
    edb1 = din("edb1", [DIM, 1])
    edW2 = din("edW2", [DIM, DIM], FPR)
    Wze = din("Wze", [DIM + 1, 906], FPR)
    emW1y = din("emW1y", [HID, 522], FPR)
    EMt = din("EMt", [MAXN1, 522], FPR)
    emW2e = din("emW2e", [522, HID], FPR)
    rankWb = din("rankWb", [128, DIM])
    ident = din("ident", [128, 128], FPR)
    I32r = din("I32r", [128, NSUB], FPR)
    iota113 = din("iota113", [128, 1])
    iota17 = din("iota17", [MAXN1, 1])
    S8 = din("S8", [128, 8], FPR)
    iota8_0 = din("iota8_0", [8, 1])
    iota8_1 = din("iota8_1", [8, 1])
    LT0 = din("LT0", [128, 1])
    LT1 = din("LT1", [128, 1])
    onesr = din("onesr", [1, 2 * CS], FPR)
    identF = din("identF", [128, 128])
    iotaTok = din("iotaTok", [128, 1])
    iotaQ0 = din("iotaQ0", [128, 1])
    iotaQ1 = din("iotaQ1", [128, 1])

    z_d = nc.dram_tensor("z_out", [S, HID], FP, kind="ExternalOutput").ap()

    with tile.TileContext(nc) as tc, ExitStack() as ctx, \
            nc.allow_low_precision(reason="f32r stores fp32 bits"):
        wpool = ctx.enter_context(tc.tile_pool(name="wpool", bufs=1))
        rk8 = ctx.enter_context(tc.tile_pool(name="rk8", bufs=1))
        glob = ctx.enter_context(tc.tile_pool(name="glob", bufs=2))
        work = ctx.enter_context(tc.tile_pool(name="work", bufs=2))
        work3 = ctx.enter_context(tc.tile_pool(name="work3", bufs=2))
        ps = ctx.enter_context(tc.tile_pool(name="ps", bufs=1, space="PSUM"))
        dstg = ctx.enter_context(
            tc.tile_pool(name="dstg", bufs=4, space="DRAM"))
        xstg = ctx.enter_context(
            tc.tile_pool(name="xstg", bufs=1, space="DRAM"))

        def wload(ap, name):
            """Load a [K, M] weight as a list of <=128-partition K-slabs."""
            k = ap.shape[0]
            if len(ap.shape) == 1 or k <= 128:
                t = wpool.tile(ap.shape, ap.dtype, name=f"w_{name}")
                nc.sync.dma_start(out=t, in_=ap)
                return t
            slabs = []
            for si, (k0, k1) in enumerate(_ksplit(k)):
                t = wpool.tile([k1 - k0] + list(ap.shape[1:]), ap.dtype,
                               name=f"w_{name}_{si}")
                nc.sync.dma_start(out=t, in_=ap[k0:k1])
                slabs.append(t)
            return slabs

        s_vdW1 = wload(vdW1, "vdW1")
        s_b1d = wload(b1d, "b1d")
        s_vdW2 = wload(vdW2, "vdW2")
        s_b2d = wload(b2d, "b2d")
        s_vmW1x = wload(vmW1x, "vmW1x")
        s_b1v = wload(b1v, "b1v")
        s_vmW2 = wload(vmW2, "vmW2")
        s_b2v = wload(b2v, "b2v")
        s_Kds = wload(Kds, "Kds")
        s_KRt = wload(KRt, "KRt")
        s_kmW2e = wload(kmW2e, "kmW2e")
        s_edW1 = wload(edW1, "edW1")
        s_edb1 = wload(edb1, "edb1")
        s_edW2 = wload(edW2, "edW2")
        s_Wze = wload(Wze, "Wze")
        s_emW1y = wload(emW1y, "emW1y")
        s_EMt = wload(EMt, "EMt")
        s_emW2e = wload(emW2e, "emW2e")
        s_rankWb = wload(rankWb, "rankWb")
        s_ident = wload(ident, "ident")
        s_I32r = wload(I32r, "I32r")
        s_iota113 = wload(iota113, "iota113")
        s_iota17 = wload(iota17, "iota17")
        s_S8 = wload(S8, "S8")
        s_iota8_0 = wload(iota8_0, "iota8_0")
        s_iota8_1 = wload(iota8_1, "iota8_1")
        s_LT0 = wload(LT0, "LT0")
        s_LT1 = wload(LT1, "LT1")
        s_ones = wload(onesr, "onesr")
        s_identF = wload(identF, "identF")
        s_iotaTok = wload(iotaTok, "iotaTok")
        s_iotaQ0 = wload(iotaQ0, "iotaQ0")
        s_iotaQ1 = wload(iotaQ1, "iotaQ1")

        def psum(name, shape=(128, NSUB), tag="mmA", bufs=3, dtype=FP):
            return ps.tile(list(shape), dtype, name=name, tag=tag, bufs=bufs)

        def acopy(out, in_):
            nc.scalar.activation(out, in_, AF.Copy)

        def mm_acc(pt, slabs, msl, rhs_parts, extra=None, dtype=FPR,
                   keep_open=False):
            """pt += W[:, msl].T @ rhs for one <=128-wide M slice `msl`."""
            if not isinstance(slabs, list):
                slabs = [slabs]
            assert len(slabs) == len(rhs_parts)
            nk = len(rhs_parts) + (1 if extra is not None else 0)
            if keep_open:
                nk = len(rhs_parts) + 1
            for i, (sl, rp) in enumerate(zip(slabs, rhs_parts)):
                assert sl.shape[0] == rp.shape[0], (sl.shape, rp.shape)
                nc.tensor.matmul(
                    pt, sl[:, msl].bitcast(dtype), rp.bitcast(dtype),
                    start=(i == 0), stop=(i == nk - 1))
            if extra is not None:
                l2, r2 = extra
                nc.tensor.matmul(
                    pt, l2.bitcast(dtype), r2.bitcast(dtype),
                    start=False, stop=True)

        # token-major chunk view: x_r[c][p, A, d] = x[c*2048 + A*128 + p, d]
        x_r = x_d.rearrange("(c a p) d -> c p a d", c=nchunks, p=128)

        # ---- persistent per-chunk rank products ----
        OHs = [rk8.tile([128, NSUB], FPR, name=f"OH{c}")
               for c in range(nchunks)]
        OHn_all = rk8.tile([MAXN1, CS * nchunks], FPR, name="OHn_all")
        # DRAM staging for feature-major x (written pass A, read pass B)
        xT = [[xstg.tile([128, CT], FP, name=f"xT_{c}_{h}")
               for h in range(2)] for c in range(nchunks)]

        # ---- cross-step state (python handles to live tiles) ----
        st = {}

        def xtm_load(c):
            ts = []
            for hf in range(2):
                t = work3.tile([128, 8, DIM], FP, name="x_tm")
                nc.scalar.dma_start(out=t, in_=x_r[c][:, 8 * hf:8 * hf + 8])
                ts.append(t)
            st[("xtm", c)] = ts

        def xfm_load(c):
            fm = [work.tile([128, CT], FP, name=f"x_fm{h}") for h in (0, 1)]
            for h in range(2):
                nc.scalar.dma_start(out=fm[h], in_=xT[c][h])
            st[("xfm", c)] = fm

        def rankA(c):
            """Rank scores + mask + magd write; x transposes -> DRAM."""
            x_tm = st.pop(("xtm", c))
            s0 = c * CS
            mag16 = work3.tile([128, 16], FP, name="mag16")
            junk = work3.tile([128, DIM], FP, name="junk", bufs=1)
            for A in range(16):
                nc.vector.scalar_tensor_tensor(
                    out=junk, in0=x_tm[A // 8][:, A % 8, :], scalar=0.0,
                    in1=s_rankWb, op0=OP.bypass, op1=OP.mult,
                    accum_out=mag16[:, A:A + 1])
            magd = dstg.tile([CT], FP, name="magd")
            nc.sync.dma_start(
                out=magd.rearrange("(a p) -> p a", p=128), in_=mag16)
            st[("magd", c)] = magd
            # transposes to feature-major, staged out to DRAM
            for ns in range(4):
                for h in range(2):
                    pt = psum("pt", (128, 4, 128), tag="tr", bufs=2, dtype=FP)
                    for a in range(4):
                        A = 4 * ns + a
                        nc.tensor.transpose(
                            pt[:, a, :],
                            x_tm[A // 8][:, A % 8, 128 * h:128 * (h + 1)],
                            s_identF)
                    stg = work3.tile([128, 4, 128], FP, name="xs_stg")
                    if h == 0:
                        acopy(stg, pt)
                    else:
                        nc.vector.tensor_copy(out=stg, in_=pt)
                    nc.scalar.dma_start(
                        out=xT[c][h][:, NSUB * ns:NSUB * (ns + 1)]
                        .rearrange("p (a q) -> p a q", a=4),
                        in_=stg)

        def rankB(c):
            """Comparison-count ranks, OH / OHn one-hots."""
            s0 = c * CS
            magd = st.pop(("magd", c))
            magdv = magd.rearrange("(s i) -> i s", i=N)
            n_i32 = glob.tile([128, CS], I32, name="n_i32", bufs=1)
            nc.sync.dma_start(
                out=n_i32,
                in_=n_d[s0:s0 + CS].unsqueeze(0).broadcast_to([128, CS]))
            n_repf = glob.tile([128, CS], FP, name="n_repf", bufs=1)
            nc.vector.tensor_copy(out=n_repf, in_=n_i32)
            # mask padded slots to +BIG in [i, s] layout, restage i-major
            mag_fm = glob.tile([N, CS], FP, name="mag_fm", bufs=1)
            nc.sync.dma_start(out=mag_fm, in_=magdv)
            inv = glob.tile([N, CS], FP, name="inv", bufs=1)
            nc.vector.tensor_scalar(
                out=inv, in0=n_repf[0:N], scalar1=s_iota17[0:N], scalar2=None,
                op0=OP.is_le)
            mag_m = glob.tile([N, CS], FP, name="mag_m", bufs=1)
            nc.vector.scalar_tensor_tensor(
                out=mag_m, in0=inv, scalar=BIG, in1=mag_fm,
                op0=OP.mult, op1=OP.add)
            magd2 = dstg.tile([N * CS], FP, name="magd2")
            nc.sync.dma_start(out=magd2, in_=mag_m)
            magd2v = magd2.rearrange("(i s) -> i s", s=CS)
            X2 = glob.tile([128, CS], FP, name="X2", bufs=1)
            nc.sync.dma_start(
                out=X2, in_=magd2v.unsqueeze(0).broadcast_to([8, N, CS]))
            rankd = dstg.tile([CT], FPR, name="rankd")
            rankdT = rankd.rearrange("(s i) -> i s", i=N)
            for h in range(2):
                X1 = glob.tile([128, CS], FP, name="X1", bufs=1)
                nc.sync.dma_start(
                    out=X1,
                    in_=magd2v[8 * h:8 * h + 8, :].unsqueeze(1)
                    .broadcast_to([8, N, CS]))
                cmp = glob.tile([128, CS], FPR, name="cmp", bufs=1)
                eq = glob.tile([128, CS], FP, name="eq", bufs=1)
                nc.vector.tensor_tensor(
                    out=cmp, in0=X2, in1=X1, op=OP.is_lt)
                nc.vector.tensor_tensor(
                    out=eq, in0=X2, in1=X1, op=OP.is_equal)
                nc.vector.scalar_tensor_tensor(
                    out=cmp, in0=eq, scalar=(s_LT0 if h == 0 else s_LT1),
                    in1=cmp, op0=OP.mult, op1=OP.add)
                pr = psum("pr", (8, CS), tag="sm", bufs=1)
                nc.tensor.matmul(pr, s_S8, cmp)
                rh = glob.tile([8, CS], FP, name=f"rh{h}", bufs=1)
                nc.vector.tensor_copy(out=rh, in_=pr)
                # rank_m = rank + inv * (16 - rank), per 8-row half
                ih = glob.tile([8, CS], FP, name=f"ih{h}", bufs=1)
                nc.vector.tensor_scalar(
                    out=ih, in0=n_repf[0:8],
                    scalar1=(s_iota8_0 if h == 0 else s_iota8_1),
                    scalar2=None, op0=OP.is_le)
                th = glob.tile([8, CS], FPR, name=f"th{h}", bufs=1)
                nc.vector.tensor_scalar(
                    out=th, in0=rh, scalar1=-1.0, scalar2=16.0,
                    op0=OP.mult, op1=OP.add)
                nc.vector.tensor_tensor(
                    out=th, in0=th.bitcast(FP), in1=ih, op=OP.mult)
                nc.vector.tensor_tensor(
                    out=th, in0=th.bitcast(FP), in1=rh, op=OP.add)
                nc.sync.dma_start(out=rankdT[8 * h:8 * h + 8, :], in_=th)
            # one-hot tile: OH[32g + r, tok_of_subchunk_g] = (rank == r)
            OH = OHs[c]
            for g in range(4):
                nc.sync.dma_start(
                    out=OH[32 * g:32 * (g + 1), :],
                    in_=rankd[NSUB * g:NSUB * (g + 1)].unsqueeze(0)
                    .broadcast_to([32, NSUB]))
            nc.vector.tensor_scalar(
                out=OH, in0=OH.bitcast(FP), scalar1=s_iota113, scalar2=None,
                op0=OP.is_equal)
            # n one-hot for the em MLP
            nc.vector.tensor_scalar(
                out=OHn_all[:, CS * c:CS * (c + 1)], in0=n_repf[0:MAXN1],
                scalar1=s_iota17, scalar2=None, op0=OP.is_equal)

        def deepset(c):
            x_fm = st[("xfm", c)]
            half = CS * (c % 2)
            if c % 2 == 0:
                st[("y2ds", c // 2)] = [
                    glob.tile([128, 2 * CS], FPR, name=f"y2ds{m}")
                    for m in (0, 1)]
            y2ds = st[("y2ds", c // 2)]
            OH = OHs[c]
            for ns in range(4):
                tsl = slice(NSUB * ns, NSUB * (ns + 1))
                xp = [x_fm[0][:, tsl], x_fm[1][:, tsl]]
                oh = OH[32 * ns:32 * ns + MAXN1, :]
                Hd = []
                for m in range(2):
                    pd = psum(f"pd{m}")
                    mm_acc(pd, s_vdW1, slice(128 * m, 128 * (m + 1)), xp)
                    hd = work3.tile([128, NSUB], FPR, name=f"Hd{m}", bufs=1)
                    nc.scalar.activation(hd, pd, AF.Relu, bias=s_b1d[m])
                    Hd.append(hd)
                for m in range(2):
                    pg = psum(f"pg{m}", tag="mmB", bufs=2)
                    nc.tensor.matmul(
                        pg,
                        s_Kds[32 * ns:32 * ns + MAXN1,
                              128 * m:128 * (m + 1)],
                        oh, tile_position=(32 * ns, 0))
                    kg = work3.tile([128, NSUB], FP, name="KG", bufs=1)
                    nc.vector.tensor_copy(out=kg, in_=pg)
                    pv = psum(f"pv{m}")
                    mm_acc(pv, s_vdW2, slice(128 * m, 128 * (m + 1)), Hd)
                    pds = psum(f"Pds{m}", tag="mmB", bufs=2)
                    nc.vector.scalar_tensor_tensor(
                        out=pds, in0=pv, scalar=s_b2d[m],
                        in1=kg, op0=OP.add, op1=OP.mult)
                    nc.vector.tensor_reduce(
                        out=y2ds[m][:, half + 32 * ns:half + 32 * (ns + 1)],
                        in_=pds.rearrange("p (s i) -> p s i", i=N),
                        axis=mybir.AxisListType.X, op=OP.add)

        def edpair(k):
            """ed MLP + z projections for chunk pair (2k, 2k+1)."""
            y2ds = st.pop(("y2ds", k))
            He = []
            for m in range(2):
                pe = psum(f"pe{m}", (128, 2 * CS))
                mm_acc(pe, s_edW1, slice(128 * m, 128 * (m + 1)), y2ds)
                he = glob.tile([128, 2 * CS], FPR, name=f"He{m}", bufs=1)
                nc.scalar.activation(he, pe, AF.Relu, bias=s_edb1[m])
                He.append(he)
            ze = []
            for m in range(2):
                pz = psum(f"pz{m}", (128, 2 * CS), tag="mmB", bufs=2)
                mm_acc(pz, s_edW2, slice(128 * m, 128 * (m + 1)), He)
                z1 = glob.tile([128, 2 * CS], FPR, name=f"ze{m}", bufs=1)
                acopy(z1, pz)
                ze.append(z1)
            for cp in range(2):
                c = 2 * k + cp
                csl2 = slice(CS * cp, CS * (cp + 1))
                zpT_s = glob.tile([128, 906], FPR, name="zpT_s")
                for half, csl in ((0, slice(0, 452)), (1, slice(452, 906))):
                    w = csl.stop - csl.start
                    pzt = psum("pzt", (128, 454), tag="mmB", bufs=2)
                    mm_acc(pzt[:, :w], [ze[0][:, csl2], ze[1][:, csl2],
                                        s_ones[:, 0:CS]], slice(None),
                           [sw[:, csl] for sw in s_Wze])
                    acopy(zpT_s[:, csl], pzt[:, :w])
                st[("zpT", c)] = zpT_s

        def mainphase(c):
            x_fm = st.pop(("xfm", c))
            zpT_s = st.pop(("zpT", c))
            half = CS * (c % 2)
            if c % 2 == 0:
                st[("y2m", c // 2)] = [
                    glob.tile([128, 2 * CS], FPR, name=f"y2m{m}")
                    for m in range(4)]
            y2m = st[("y2m", c // 2)]
            OH = OHs[c]
            for ns in range(4):
                tsl = slice(NSUB * ns, NSUB * (ns + 1))
                xp = [x_fm[0][:, tsl], x_fm[1][:, tsl]]
                oh = OH[32 * ns:32 * ns + MAXN1, :]
                i32 = s_I32r[32 * ns:32 * (ns + 1), :]
                zsl = slice(32 * ns, 32 * (ns + 1))
                Hv = []
                for m in range(4):
                    pvm = psum(f"pvm{m}")
                    mm_acc(pvm, s_vmW1x, slice(128 * m, 128 * (m + 1)),
                           xp, keep_open=True)
                    nc.tensor.matmul(
                        pvm, zpT_s[zsl, 128 * m:128 * (m + 1)],
                        i32, start=False, stop=True,
                        tile_position=(32 * ns, 0))
                    hv = work3.tile([128, NSUB], FPR, name=f"Hv{m}", bufs=1)
                    nc.scalar.activation(hv, pvm, AF.Relu, bias=s_b1v[m])
                    Hv.append(hv)
                Hk = []
                for m in range(4):
                    mw = 128 if m < 3 else 10
                    pkm = psum(f"pkm{m}")
                    nc.tensor.matmul(
                        pkm[:mw, :] if mw != 128 else pkm,
                        s_KRt[32 * ns:32 * ns + MAXN1,
                              128 * m:128 * m + mw],
                        oh, start=True, stop=False,
                        tile_position=(32 * ns, 0))
                    nc.tensor.matmul(
                        pkm[:mw, :] if mw != 128 else pkm,
                        zpT_s[zsl, 512 + 128 * m:512 + 128 * m + mw],
                        i32, start=False, stop=True,
                        tile_position=(32 * ns, 0))
                    hk = work3.tile([mw, NSUB], FPR, name=f"Hk{m}", bufs=1)
                    nc.scalar.activation(
                        hk, pkm[:mw, :] if mw != 128 else pkm, AF.Relu)
                    Hk.append(hk)
                for m in range(4):
                    pK = psum(f"pK{m}", tag="mmB", bufs=2)
                    mm_acc(pK, s_kmW2e, slice(128 * m, 128 * (m + 1)), Hk)
                    km = work3.tile([128, NSUB], FP, name="Km", bufs=1)
                    acopy(km, pK)
                    pV = psum(f"pV{m}")
                    mm_acc(pV, s_vmW2, slice(128 * m, 128 * (m + 1)), Hv)
                    pmt = psum(f"Pm{m}", tag="mmB", bufs=2)
                    nc.vector.scalar_tensor_tensor(
                        out=pmt, in0=pV, scalar=s_b2v[m],
                        in1=km, op0=OP.add, op1=OP.mult)
                    nc.vector.tensor_reduce(
                        out=y2m[m][:, half + 32 * ns:half + 32 * (ns + 1)],
                        in_=pmt.rearrange("p (s i) -> p s i", i=N),
                        axis=mybir.AxisListType.X, op=OP.add)

        def empair(k):
            """em MLP + output for chunk pair (2k, 2k+1)."""
            y2m = st.pop(("y2m", k))
            ohn = OHn_all[:, 2 * CS * k:2 * CS * (k + 1)]
            Hm = []
            for m in range(5):
                mw = 128 if m < 4 else 10
                pem = psum(f"pem{m}", (128, 2 * CS))
                pem_v = pem[:mw, :] if mw != 128 else pem
                mm_acc(pem_v, s_emW1y, slice(128 * m, 128 * m + mw), y2m,
                       extra=(s_EMt[:, 128 * m:128 * m + mw], ohn))
                hm = glob.tile([mw, 2 * CS], FPR, name=f"Hm{m}", bufs=1)
                nc.scalar.activation(hm, pem_v, AF.Relu)
                Hm.append(hm)
            zo = []
            for m in range(4):
                pzo = psum(f"pzo{m}", (128, 2 * CS), tag="mmB", bufs=2)
                mm_acc(pzo, s_emW2e, slice(128 * m, 128 * (m + 1)), Hm)
                z1 = glob.tile([128, 2 * CS], FPR, name=f"zo{m}", bufs=1)
                acopy(z1, pzo)
                zo.append(z1)
            for cp in range(2):
                s0 = (2 * k + cp) * CS
                zt = psum("zt", (128, 4, 128), tag="tr", bufs=2, dtype=FPR)
                for m in range(4):
                    nc.tensor.transpose(
                        zt[:, m, :], zo[m][:, CS * cp:CS * (cp + 1)],
                        s_ident)
                z_tm = glob.tile([128, 4, 128], FP, name="z_tm", bufs=1)
                nc.vector.tensor_copy(out=z_tm, in_=zt.bitcast(FP))
                nc.scalar.dma_start(
                    out=z_d[s0:s0 + CS, :].rearrange("s (m f) -> s m f", m=4),
                    in_=z_tm)

        # ---------------- schedule ----------------
        xtm_load(0)
        rankA(0)
        xtm_load(1)
        rankB(0)
        rankA(1)
        xtm_load(2)
        rankB(1)
        rankA(2)
        rankB(2)
        xtm_load(3)
        xfm_load(0)
        for c in range(nchunks):
            deepset(c)
            if c + 3 < nchunks:
                rankA(c + 3)
            if c % 2 == 1:
                edpair((c - 1) // 2)
            if c >= 1:
                mainphase(c - 1)
            if c % 2 == 1 and c >= 3:
                empair((c - 3) // 2)
            if c + 3 < nchunks:
                rankB(c + 3)
            if c + 4 < nchunks:
                xtm_load(c + 4)
            if c + 1 < nchunks:
                # issued after mainphase(c-1) so the bufs=2 WAR rotation on
                # x_fm is against already-issued readers
                xfm_load(c + 1)
        mainphase(nchunks - 1)
        empair(nchunks // 2 - 1)

    nc.compile()
    return nc


def make_tables(inp):
    """Host-side weight preprocessing -> dict of extra input arrays."""
    f = np.float32
    keys = ("rank_W", "kd_W1", "kd_b1", "kd_W2", "kd_b2", "vd_W1", "vd_b1",
            "vd_W2", "vd_b2", "ed_W1", "ed_b1", "ed_W2", "ed_b2", "km_W1",
            "km_b1", "km_W2", "km_b2", "vm_W1", "vm_b1", "vm_W2", "vm_b2",
            "em_W1", "em_b1", "em_W2", "em_b2")
    g = {k: np.asarray(inp[k], f) for k in keys}

    def A(v):
        return np.ascontiguousarray(v, dtype=f)

    kd_h = np.maximum(g["kd_W1"][:16] + g["kd_b1"][None, :], 0.0)
    Kds16 = kd_h @ g["kd_W2"] + g["kd_b2"][None, :]
    Kds17 = np.vstack([Kds16, np.zeros((1, DIM), f)])
    # km first-layer position rows; extra col 392 = valid-mask generator
    # (one-hot rhs sums to 1 -> relu(col392 row) == mask row)
    KRt17 = np.vstack([g["km_W1"][:16] + g["km_b1"][None, :],
                       np.full((1, 392), -BIG, f)])
    KRt17 = np.hstack([KRt17,
                       np.concatenate([np.ones((16, 1), f),
                                       np.zeros((1, 1), f)]),
                       np.zeros((MAXN1, 1), f)])

    def rep4(tab):
        out = np.zeros((128, tab.shape[1]), f)
        for gi in range(4):
            out[32 * gi:32 * gi + MAXN1] = tab
        return out

    Kds = rep4(Kds17)
    KRt = rep4(KRt17)
    kmW2e = np.vstack([g["km_W2"], g["km_b2"][None, :],
                       np.zeros((1, HID), f)])

    Wz = np.hstack([g["vm_W1"][DIM:2 * DIM], g["km_W1"][MAXN1:MAXN1 + DIM],
                    np.zeros((DIM, 2), f)])
    Wze = np.vstack([Wz, (g["ed_b2"] @ Wz)[None, :]])

    # em first layer; extra col 520 = ones generator via the n one-hot
    EMt = np.hstack([g["em_W1"][HID:HID + MAXN1] + g["em_b1"][None, :],
                     np.ones((MAXN1, 1), f), np.zeros((MAXN1, 1), f)])
    emW1y = np.hstack([g["em_W1"][:HID], np.zeros((HID, 2), f)])
    emW2e = np.vstack([g["em_W2"], g["em_b2"][None, :],
                       np.zeros((1, HID), f)])

    p = np.arange(128)
    iota113 = np.where(p % 32 < MAXN1, p % 32, 99).astype(f)[:, None]
    t = np.arange(NSUB)
    i32g = (t[None, :] // N == np.arange(32)[:, None]).astype(f)
    I32r = np.zeros((128, NSUB), f)
    for gi in range(4):
        I32r[32 * gi:32 * (gi + 1)] = i32g
    S8 = (p[:, None] // 16 == np.arange(8)[None, :]).astype(f)
    LT0 = ((p % 16) < (p // 16)).astype(f)[:, None]
    LT1 = ((p % 16) < (p // 16 + 8)).astype(f)[:, None]

    return {
        "vdW1": A(g["vd_W1"]), "b1d": A(g["vd_b1"][:, None]),
        "vdW2": A(g["vd_W2"]), "b2d": A(g["vd_b2"][:, None]),
        "vmW1x": A(g["vm_W1"][:DIM]), "b1v": A(g["vm_b1"][:, None]),
        "vmW2": A(g["vm_W2"]), "b2v": A(g["vm_b2"][:, None]),
        "Kds": A(Kds), "KRt": A(KRt), "kmW2e": A(kmW2e),
        "edW1": A(g["ed_W1"]), "edb1": A(g["ed_b1"][:, None]),
        "edW2": A(g["ed_W2"]), "Wze": A(Wze),
        "emW1y": A(emW1y), "EMt": A(EMt), "emW2e": A(emW2e),
        "rankWb": A(np.tile(g["rank_W"].T, (128, 1))),
        "ident": A(np.eye(128)), "I32r": A(I32r),
        "iota113": A(iota113),
        "iota17": A(np.arange(MAXN1)[:, None]),
        "S8": A(S8), "LT0": A(LT0), "LT1": A(LT1),
        "onesr": A(np.ones((1, 2 * CS))),
        "identF": A(np.eye(128)),
        "iota8_0": A(np.arange(8)[:, None]),
        "iota8_1": A(np.arange(8, 16)[:, None]),
        "iotaTok": A((p % 16).astype(f)[:, None]),
        "iotaQ0": A((p // 16).astype(f)[:, None]),
        "iotaQ1": A((8 + p // 16).astype(f)[:, None]),
    }


_prog_cache = {}


def _get_program(sets_per_core, num_devices):
    key = (sets_per_core, num_devices)
    if key not in _prog_cache:
        _prog_cache[key] = build_program(sets_per_core, num_devices)
    return _prog_cache[key]


def kernel(**inputs):
    nc = _get_program(SC, NCORES)
    tabs = make_tables(inputs)
    x = np.ascontiguousarray(np.asarray(inputs["x"], np.float32))
    n = np.ascontiguousarray(np.asarray(inputs["n"], np.int32))
    in_maps = []
    for c in range(NCORES):
        m = dict(tabs)
        m["x"] = np.ascontiguousarray(
            x[c * SC:(c + 1) * SC].reshape(SC * N, DIM))
        m["n_i"] = n[c * SC:(c + 1) * SC]
        in_maps.append(m)
    res = bass_utils.run_bass_kernel_spmd(nc, in_maps, list(range(NCORES)))
    z = np.concatenate([res.results[c]["z_out"] for c in range(NCORES)], 0)
    return z


# revision 30
# speedup vs baseline: 1.4141x; 1.1273x over previous
"""Trainium2 Bass kernel for nn_Encoder_62740882260638 (ragged set encoder).

Pure data parallel over 8 NeuronCores: each core handles B/8 = 1024 of the
8192 sets; weights replicated; no collectives.

Structure (v2 — software-pipelined two-pass):
  * Pass A (rank pipeline, issued 3 chunks ahead of pass B): loads x
    token-major, computes rank scores on DVE (exact fp32 accumulate ->
    matches the jax sort), masks padded slots, runs the comparison-count
    rank via small PE matmuls, builds the rank one-hot OH and the n one-hot
    OHn, and PE-transposes x to feature-major, staged to DRAM.
  * Pass B (per chunk): deepset branch, ed MLP (chunk-PAIRED so matmuls run
    256 cols — fp32r needs >=256 cols for 1 cycle/row), main branch, em MLP
    (also paired).  Issue order is software-pipelined:
    deepset(c) | rankA(c+3) | edpair | main(c-1) | rankB(c+3) | empair
    so the in-order PE queue never waits on the rank pipeline's DMA
    roundtrips and stays continuously busy (p-state ramp to 2.4 GHz).
  * Ragged masking is free: padded tokens get rank 16 -> routed to trash
    table rows (0 for the deepset key table; -1e30 for the km rows so relu
    zeroes the hidden).  The km mask row and em ones row are generated by
    EXTRA TABLE COLUMNS (a one-hot rhs sums to 1), not DMAs.
  * Engine split: PE matmuls; Act relus + psum copies + bulk DMA queue;
    DVE rank ops + product STTs; GpSimd segmented reduces + psum copies.
"""

from contextlib import ExitStack

import numpy as np

import concourse.bass as bass
import concourse.mybir as mybir
import concourse.tile as tile
from concourse import bacc
from concourse import bass_utils

FP = mybir.dt.float32
FPR = mybir.dt.float32r
BF = mybir.dt.bfloat16
I32 = mybir.dt.int32
AF = mybir.ActivationFunctionType
OP = mybir.AluOpType

B, N, DIM, HID, MAXN1 = 8192, 16, 256, 512, 17
NCORES = 8
SC = B // NCORES  # sets per core (1024)
CS = 128          # sets per chunk
CT = CS * N       # tokens per chunk (2048)
NSUB = 512        # tokens per sub-chunk (matmul N)
BIG = 1.0e30


def _ksplit(total):
    return [(a, min(a + 128, total)) for a in range(0, total, 128)]


def build_program(sets_per_core=SC, num_devices=1):
    nc = bacc.Bacc(
        "TRN2", target_bir_lowering=False, debug=False,
        num_devices=num_devices,
    )
    S = sets_per_core
    assert S % (2 * CS) == 0
    nchunks = S // CS

    def din(name, shape, dtype=FP):
        return nc.dram_tensor(name, shape, dtype, kind="ExternalInput").ap()

    x_d = din("x", [S * N, DIM])
    n_d = din("n_i", [S], I32)

    vdW1 = din("vdW1", [DIM, DIM], BF)
    b1d = din("b1d", [DIM, 1])
    vdW2 = din("vdW2", [DIM, DIM], FPR)
    b2d = din("b2d", [DIM, 1])
    vmW1x = din("vmW1x", [DIM, HID], BF)
    b1v = din("b1v", [HID, 1])
    vmW2 = din("vmW2", [HID, HID], BF)
    b2v = din("b2v", [HID, 1])
    Kds = din("Kds", [128, DIM], FPR)
    KRt = din("KRt", [128, 394], FPR)
    kmW2e = din("kmW2e", [394, HID], BF)
    edW1 = din("edW1", [DIM, DIM], FPR)
    edb1 = din("edb1", [DIM, 1])
    edW2 = din("edW2", [DIM, DIM], FPR)
    Wze = din("Wze", [DIM + 1, 906], FPR)
    emW1y = din("emW1y", [HID, 522], FPR)
    EMt = din("EMt", [MAXN1, 522], FPR)
    emW2e = din("emW2e", [522, HID], FPR)
    rankWb = din("rankWb", [128, DIM])
    ident = din("ident", [128, 128], FPR)
    I32r = din("I32r", [128, NSUB], FPR)
    iota113 = din("iota113", [128, 1])
    iota17 = din("iota17", [MAXN1, 1])
    S8 = din("S8", [128, 8], FPR)
    iota8_0 = din("iota8_0", [8, 1])
    iota8_1 = din("iota8_1", [8, 1])
    LT0 = din("LT0", [128, 1])
    LT1 = din("LT1", [128, 1])
    onesr = din("onesr", [1, 2 * CS], FPR)
    identF = din("identF", [128, 128])
    iotaTok = din("iotaTok", [128, 1])
    iotaQ0 = din("iotaQ0", [128, 1])
    iotaQ1 = din("iotaQ1", [128, 1])

    z_d = nc.dram_tensor("z_out", [S, HID], FP, kind="ExternalOutput").ap()

    with tile.TileContext(nc) as tc, ExitStack() as ctx, \
            nc.allow_low_precision(reason="f32r stores fp32 bits"):
        wpool = ctx.enter_context(tc.tile_pool(name="wpool", bufs=1))
        rk8 = ctx.enter_context(tc.tile_pool(name="rk8", bufs=1))
        glob = ctx.enter_context(tc.tile_pool(name="glob", bufs=2))
        work = ctx.enter_context(tc.tile_pool(name="work", bufs=2))
        work3 = ctx.enter_context(tc.tile_pool(name="work3", bufs=2))
        ps = ctx.enter_context(tc.tile_pool(name="ps", bufs=1, space="PSUM"))
        dstg = ctx.enter_context(
            tc.tile_pool(name="dstg", bufs=4, space="DRAM"))
        xstg = ctx.enter_context(
            tc.tile_pool(name="xstg", bufs=1, space="DRAM"))

        def wload(ap, name):
            """Load a [K, M] weight as a list of <=128-partition K-slabs."""
            k = ap.shape[0]
            if len(ap.shape) == 1 or k <= 128:
                t = wpool.tile(ap.shape, ap.dtype, name=f"w_{name}")
                nc.sync.dma_start(out=t, in_=ap)
                return t
            slabs = []
            for si, (k0, k1) in enumerate(_ksplit(k)):
                t = wpool.tile([k1 - k0] + list(ap.shape[1:]), ap.dtype,
                               name=f"w_{name}_{si}")
                nc.sync.dma_start(out=t, in_=ap[k0:k1])
                slabs.append(t)
            return slabs

        s_vdW1 = wload(vdW1, "vdW1")
        s_b1d = wload(b1d, "b1d")
        s_vdW2 = wload(vdW2, "vdW2")
        s_b2d = wload(b2d, "b2d")
        s_vmW1x = wload(vmW1x, "vmW1x")
        s_b1v = wload(b1v, "b1v")
        s_vmW2 = wload(vmW2, "vmW2")
        s_b2v = wload(b2v, "b2v")
        s_Kds = wload(Kds, "Kds")
        s_KRt = wload(KRt, "KRt")
        s_kmW2e = wload(kmW2e, "kmW2e")
        s_edW1 = wload(edW1, "edW1")
        s_edb1 = wload(edb1, "edb1")
        s_edW2 = wload(edW2, "edW2")
        s_Wze = wload(Wze, "Wze")
        s_emW1y = wload(emW1y, "emW1y")
        s_EMt = wload(EMt, "EMt")
        s_emW2e = wload(emW2e, "emW2e")
        s_rankWb = wload(rankWb, "rankWb")
        s_ident = wload(ident, "ident")
        s_I32r = wload(I32r, "I32r")
        s_iota113 = wload(iota113, "iota113")
        s_iota17 = wload(iota17, "iota17")
        s_S8 = wload(S8, "S8")
        s_iota8_0 = wload(iota8_0, "iota8_0")
        s_iota8_1 = wload(iota8_1, "iota8_1")
        s_LT0 = wload(LT0, "LT0")
        s_LT1 = wload(LT1, "LT1")
        s_ones = wload(onesr, "onesr")
        s_identF = wload(identF, "identF")
        s_iotaTok = wload(iotaTok, "iotaTok")
        s_iotaQ0 = wload(iotaQ0, "iotaQ0")
        s_iotaQ1 = wload(iotaQ1, "iotaQ1")

        def psum(name, shape=(128, NSUB), tag="mmA", bufs=3, dtype=FP):
            return ps.tile(list(shape), dtype, name=name, tag=tag, bufs=bufs)

        def acopy(out, in_):
            nc.scalar.activation(out, in_, AF.Copy)

        def mm_acc(pt, slabs, msl, rhs_parts, extra=None,
                   keep_open=False):
            """pt += W[:, msl].T @ rhs for one <=128-wide M slice `msl`."""
            if not isinstance(slabs, list):
                slabs = [slabs]
            assert len(slabs) == len(rhs_parts)
            nk = len(rhs_parts) + (1 if extra is not None else 0)
            if keep_open:
                nk = len(rhs_parts) + 1
            for i, (sl, rp) in enumerate(zip(slabs, rhs_parts)):
                assert sl.shape[0] == rp.shape[0], (sl.shape, rp.shape)
                nc.tensor.matmul(
                    pt, sl[:, msl], rp,
                    start=(i == 0), stop=(i == nk - 1))
            if extra is not None:
                l2, r2 = extra
                nc.tensor.matmul(
                    pt, l2, r2,
                    start=False, stop=True)

        # token-major chunk view: x_r[c][p, A, d] = x[c*2048 + A*128 + p, d]
        x_r = x_d.rearrange("(c a p) d -> c p a d", c=nchunks, p=128)

        # ---- persistent per-chunk rank products ----
        OHs = [rk8.tile([128, NSUB], FPR, name=f"OH{c}")
               for c in range(nchunks)]
        OHn_all = rk8.tile([MAXN1, CS * nchunks], FPR, name="OHn_all")
        # DRAM staging for token-major bf16 x (written pass A, read pass B
        # via DMA XBAR transpose)
        xT = [[xstg.tile([CT, 128], BF, name=f"xT_{c}_{h}")
               for h in range(2)] for c in range(nchunks)]

        # ---- cross-step state (python handles to live tiles) ----
        st = {}

        def xtm_load(c):
            ts = []
            for hf in range(2):
                t = work3.tile([128, 8, DIM], FP, name="x_tm")
                nc.scalar.dma_start(out=t, in_=x_r[c][:, 8 * hf:8 * hf + 8])
                ts.append(t)
            st[("xtm", c)] = ts

        def xfm_load(c):
            fm = [work.tile([128, CT], BF, name=f"x_fm{h}", bufs=3)
                  for h in (0, 1)]
            for h in range(2):
                nc.scalar.dma_start(out=fm[h], in_=xT[c][h], transpose=True)
            st[("xfm", c)] = fm

        def rankA(c):
            """Rank scores + mask + magd write; x transposes -> DRAM."""
            x_tm = st.pop(("xtm", c))
            s0 = c * CS
            mag16 = work3.tile([128, 16], FP, name="mag16")
            junk = work3.tile([128, DIM], FP, name="junk", bufs=1)
            for A in range(16):
                nc.vector.scalar_tensor_tensor(
                    out=junk, in0=x_tm[A // 8][:, A % 8, :], scalar=0.0,
                    in1=s_rankWb, op0=OP.bypass, op1=OP.mult,
                    accum_out=mag16[:, A:A + 1])
            magd = dstg.tile([CT], FP, name="magd")
            nc.sync.dma_start(
                out=magd.rearrange("(a p) -> p a", p=128), in_=mag16)
            st[("magd", c)] = magd
            # cast to bf16 and stage token-major halves; pass B transposes
            # via the DMA XBAR (no PE involvement)
            for hf in range(2):
                xbf = work3.tile([128, 8, DIM], BF, name="xbf")
                if hf == 0:
                    acopy(xbf, x_tm[hf])
                else:
                    nc.vector.tensor_copy(out=xbf, in_=x_tm[hf])
                for h in range(2):
                    nc.scalar.dma_start(
                        out=xT[c][h]
                        .rearrange("(a p) f -> p a f", p=128)[:, 8 * hf:8 * hf + 8, :],
                        in_=xbf[:, :, 128 * h:128 * (h + 1)])

        def rankB(c):
            """Comparison-count ranks, OH / OHn one-hots."""
            s0 = c * CS
            magd = st.pop(("magd", c))
            magdv = magd.rearrange("(s i) -> i s", i=N)
            n_i32 = glob.tile([128, CS], I32, name="n_i32", bufs=1)
            nc.sync.dma_start(
                out=n_i32,
                in_=n_d[s0:s0 + CS].unsqueeze(0).broadcast_to([128, CS]))
            n_repf = glob.tile([128, CS], FP, name="n_repf", bufs=1)
            nc.vector.tensor_copy(out=n_repf, in_=n_i32)
            # mask padded slots to +BIG in [i, s] layout, restage i-major
            mag_fm = glob.tile([N, CS], FP, name="mag_fm", bufs=1)
            nc.sync.dma_start(out=mag_fm, in_=magdv)
            inv = glob.tile([N, CS], FP, name="inv", bufs=1)
            nc.vector.tensor_scalar(
                out=inv, in0=n_repf[0:N], scalar1=s_iota17[0:N], scalar2=None,
                op0=OP.is_le)
            mag_m = glob.tile([N, CS], FP, name="mag_m", bufs=1)
            nc.vector.scalar_tensor_tensor(
                out=mag_m, in0=inv, scalar=BIG, in1=mag_fm,
                op0=OP.mult, op1=OP.add)
            magd2 = dstg.tile([N * CS], FP, name="magd2")
            nc.sync.dma_start(out=magd2, in_=mag_m)
            magd2v = magd2.rearrange("(i s) -> i s", s=CS)
            X2 = glob.tile([128, CS], FP, name="X2", bufs=1)
            nc.sync.dma_start(
                out=X2, in_=magd2v.unsqueeze(0).broadcast_to([8, N, CS]))
            rankd = dstg.tile([CT], FPR, name="rankd")
            rankdT = rankd.rearrange("(s i) -> i s", i=N)
            for h in range(2):
                X1 = glob.tile([128, CS], FP, name="X1", bufs=1)
                nc.sync.dma_start(
                    out=X1,
                    in_=magd2v[8 * h:8 * h + 8, :].unsqueeze(1)
                    .broadcast_to([8, N, CS]))
                cmp = glob.tile([128, CS], FPR, name="cmp", bufs=1)
                eq = glob.tile([128, CS], FP, name="eq", bufs=1)
                nc.vector.tensor_tensor(
                    out=cmp, in0=X2, in1=X1, op=OP.is_lt)
                nc.vector.tensor_tensor(
                    out=eq, in0=X2, in1=X1, op=OP.is_equal)
                nc.vector.scalar_tensor_tensor(
                    out=cmp, in0=eq, scalar=(s_LT0 if h == 0 else s_LT1),
                    in1=cmp, op0=OP.mult, op1=OP.add)
                pr = psum("pr", (8, CS), tag="sm", bufs=1)
                nc.tensor.matmul(pr, s_S8, cmp)
                rh = glob.tile([8, CS], FP, name=f"rh{h}", bufs=1)
                nc.vector.tensor_copy(out=rh, in_=pr)
                # rank_m = rank + inv * (16 - rank), per 8-row half
                ih = glob.tile([8, CS], FP, name=f"ih{h}", bufs=1)
                nc.vector.tensor_scalar(
                    out=ih, in0=n_repf[0:8],
                    scalar1=(s_iota8_0 if h == 0 else s_iota8_1),
                    scalar2=None, op0=OP.is_le)
                th = glob.tile([8, CS], FPR, name=f"th{h}", bufs=1)
                nc.vector.tensor_scalar(
                    out=th, in0=rh, scalar1=-1.0, scalar2=16.0,
                    op0=OP.mult, op1=OP.add)
                nc.vector.tensor_tensor(
                    out=th, in0=th.bitcast(FP), in1=ih, op=OP.mult)
                nc.vector.tensor_tensor(
                    out=th, in0=th.bitcast(FP), in1=rh, op=OP.add)
                nc.sync.dma_start(out=rankdT[8 * h:8 * h + 8, :], in_=th)
            # one-hot tile: OH[32g + r, tok_of_subchunk_g] = (rank == r)
            OH = OHs[c]
            OHf = glob.tile([128, NSUB], FP, name="OHf", bufs=1)
            for g in range(4):
                nc.sync.dma_start(
                    out=OHf[32 * g:32 * (g + 1), :].bitcast(FPR),
                    in_=rankd[NSUB * g:NSUB * (g + 1)].unsqueeze(0)
                    .broadcast_to([32, NSUB]))
            nc.vector.tensor_scalar(
                out=OH, in0=OHf, scalar1=s_iota113, scalar2=None,
                op0=OP.is_equal)
            # n one-hot for the em MLP
            nc.vector.tensor_scalar(
                out=OHn_all[:, CS * c:CS * (c + 1)], in0=n_repf[0:MAXN1],
                scalar1=s_iota17, scalar2=None, op0=OP.is_equal)

        def deepset(c):
            x_fm = st[("xfm", c)]
            half = CS * (c % 2)
            if c % 2 == 0:
                st[("y2ds", c // 2)] = [
                    glob.tile([128, 2 * CS], FPR, name=f"y2ds{m}")
                    for m in (0, 1)]
            y2ds = st[("y2ds", c // 2)]
            OH = OHs[c]
            for ns in range(4):
                tsl = slice(NSUB * ns, NSUB * (ns + 1))
                xp = [x_fm[0][:, tsl], x_fm[1][:, tsl]]
                oh = OH[32 * ns:32 * ns + MAXN1, :]
                Hd = []
                for m in range(2):
                    pd = psum(f"pd{m}")
                    mm_acc(pd, s_vdW1, slice(128 * m, 128 * (m + 1)), xp)
                    hd = work3.tile([128, NSUB], FPR, name=f"Hd{m}", bufs=1)
                    nc.scalar.activation(hd, pd, AF.Relu, bias=s_b1d[m])
                    Hd.append(hd)
                for m in range(2):
                    pg = psum(f"pg{m}", tag="mmB", bufs=2)
                    nc.tensor.matmul(
                        pg,
                        s_Kds[32 * ns:32 * ns + MAXN1,
                              128 * m:128 * (m + 1)],
                        oh, tile_position=(32 * ns, 0))
                    kg = work3.tile([128, NSUB], FP, name="KG", bufs=1)
                    nc.vector.tensor_copy(out=kg, in_=pg)
                    pv = psum(f"pv{m}")
                    mm_acc(pv, s_vdW2, slice(128 * m, 128 * (m + 1)), Hd)
                    pds = psum(f"Pds{m}", tag="mmB", bufs=2)
                    nc.vector.scalar_tensor_tensor(
                        out=pds, in0=pv, scalar=s_b2d[m],
                        in1=kg, op0=OP.add, op1=OP.mult)
                    nc.vector.tensor_reduce(
                        out=y2ds[m][:, half + 32 * ns:half + 32 * (ns + 1)],
                        in_=pds.rearrange("p (s i) -> p s i", i=N),
                        axis=mybir.AxisListType.X, op=OP.add)

        def edpair(k):
            """ed MLP + z projections for chunk pair (2k, 2k+1)."""
            y2ds = st.pop(("y2ds", k))
            He = []
            for m in range(2):
                pe = psum(f"pe{m}", (128, 2 * CS))
                mm_acc(pe, s_edW1, slice(128 * m, 128 * (m + 1)), y2ds)
                he = glob.tile([128, 2 * CS], FPR, name=f"He{m}", bufs=1)
                nc.scalar.activation(he, pe, AF.Relu, bias=s_edb1[m])
                He.append(he)
            ze = []
            for m in range(2):
                pz = psum(f"pz{m}", (128, 2 * CS), tag="mmB", bufs=2)
                mm_acc(pz, s_edW2, slice(128 * m, 128 * (m + 1)), He)
                z1 = glob.tile([128, 2 * CS], FPR, name=f"ze{m}", bufs=1)
                acopy(z1, pz)
                ze.append(z1)
            for cp in range(2):
                c = 2 * k + cp
                csl2 = slice(CS * cp, CS * (cp + 1))
                zpT_s = glob.tile([128, 906], FPR, name="zpT_s")
                for half, csl in ((0, slice(0, 452)), (1, slice(452, 906))):
                    w = csl.stop - csl.start
                    pzt = psum("pzt", (128, 454), tag="mmB", bufs=2)
                    mm_acc(pzt[:, :w], [ze[0][:, csl2], ze[1][:, csl2],
                                        s_ones[:, 0:CS]], slice(None),
                           [sw[:, csl] for sw in s_Wze])
                    acopy(zpT_s[:, csl], pzt[:, :w])
                st[("zpT", c)] = zpT_s

        def mainphase(c):
            x_fm = st.pop(("xfm", c))
            zpT_s = st.pop(("zpT", c))
            half = CS * (c % 2)
            if c % 2 == 0:
                st[("y2m", c // 2)] = [
                    glob.tile([128, 2 * CS], FPR, name=f"y2m{m}")
                    for m in range(4)]
            y2m = st[("y2m", c // 2)]
            OH = OHs[c]
            for ns in range(4):
                tsl = slice(NSUB * ns, NSUB * (ns + 1))
                xp = [x_fm[0][:, tsl], x_fm[1][:, tsl]]
                oh = OH[32 * ns:32 * ns + MAXN1, :]
                i32 = s_I32r[32 * ns:32 * (ns + 1), :]
                zsl = slice(32 * ns, 32 * (ns + 1))
                Hv = []
                for m in range(4):
                    pvm = psum(f"pvm{m}")
                    mm_acc(pvm, s_vmW1x, slice(128 * m, 128 * (m + 1)),
                           xp, keep_open=True)
                    nc.tensor.matmul(
                        pvm, zpT_s[zsl, 128 * m:128 * (m + 1)],
                        i32, start=False, stop=True,
                        tile_position=(32 * ns, 0))
                    hv = work3.tile([128, NSUB], BF, name=f"Hv{m}", bufs=1)
                    nc.scalar.activation(hv, pvm, AF.Relu, bias=s_b1v[m])
                    Hv.append(hv)
                Hk = []
                for m in range(4):
                    mw = 128 if m < 3 else 10
                    pkm = psum(f"pkm{m}")
                    nc.tensor.matmul(
                        pkm[:mw, :] if mw != 128 else pkm,
                        s_KRt[32 * ns:32 * ns + MAXN1,
                              128 * m:128 * m + mw],
                        oh, start=True, stop=False,
                        tile_position=(32 * ns, 0))
                    nc.tensor.matmul(
                        pkm[:mw, :] if mw != 128 else pkm,
                        zpT_s[zsl, 512 + 128 * m:512 + 128 * m + mw],
                        i32, start=False, stop=True,
                        tile_position=(32 * ns, 0))
                    hk = work3.tile([mw, NSUB], BF, name=f"Hk{m}", bufs=1)
                    nc.scalar.activation(
                        hk, pkm[:mw, :] if mw != 128 else pkm, AF.Relu)
                    Hk.append(hk)
                for m in range(4):
                    pK = psum(f"pK{m}", tag="mmB", bufs=2)
                    mm_acc(pK, s_kmW2e, slice(128 * m, 128 * (m + 1)), Hk)
                    km = work3.tile([128, NSUB], FP, name="Km", bufs=1)
                    acopy(km, pK)
                    pV = psum(f"pV{m}")
                    mm_acc(pV, s_vmW2, slice(128 * m, 128 * (m + 1)), Hv)
                    pmt = psum(f"Pm{m}", tag="mmB", bufs=2)
                    nc.vector.scalar_tensor_tensor(
                        out=pmt, in0=pV, scalar=s_b2v[m],
                        in1=km, op0=OP.add, op1=OP.mult)
                    nc.vector.tensor_reduce(
                        out=y2m[m][:, half + 32 * ns:half + 32 * (ns + 1)],
                        in_=pmt.rearrange("p (s i) -> p s i", i=N),
                        axis=mybir.AxisListType.X, op=OP.add)

        def empair(k):
            """em MLP + output for chunk pair (2k, 2k+1)."""
            y2m = st.pop(("y2m", k))
            ohn = OHn_all[:, 2 * CS * k:2 * CS * (k + 1)]
            Hm = []
            for m in range(5):
                mw = 128 if m < 4 else 10
                pem = psum(f"pem{m}", (128, 2 * CS))
                pem_v = pem[:mw, :] if mw != 128 else pem
                mm_acc(pem_v, s_emW1y, slice(128 * m, 128 * m + mw), y2m,
                       extra=(s_EMt[:, 128 * m:128 * m + mw], ohn))
                hm = glob.tile([mw, 2 * CS], FPR, name=f"Hm{m}", bufs=1)
                nc.scalar.activation(hm, pem_v, AF.Relu)
                Hm.append(hm)
            zo = []
            for m in range(4):
                pzo = psum(f"pzo{m}", (128, 2 * CS), tag="mmB", bufs=2)
                mm_acc(pzo, s_emW2e, slice(128 * m, 128 * (m + 1)), Hm)
                z1 = glob.tile([128, 2 * CS], FPR, name=f"zo{m}", bufs=1)
                acopy(z1, pzo)
                zo.append(z1)
            for cp in range(2):
                s0 = (2 * k + cp) * CS
                zt = psum("zt", (128, 4, 128), tag="tr", bufs=2, dtype=FPR)
                for m in range(4):
                    nc.tensor.transpose(
                        zt[:, m, :], zo[m][:, CS * cp:CS * (cp + 1)],
                        s_ident)
                z_tm = glob.tile([128, 4, 128], FP, name="z_tm", bufs=1)
                nc.vector.tensor_copy(out=z_tm, in_=zt.bitcast(FP))
                nc.scalar.dma_start(
                    out=z_d[s0:s0 + CS, :].rearrange("s (m f) -> s m f", m=4),
                    in_=z_tm)

        # ---------------- schedule ----------------
        xtm_load(0)
        rankA(0)
        rankB(0)
        xfm_load(0)
        xtm_load(1)
        rankA(1)
        rankB(1)
        xfm_load(1)
        xtm_load(2)
        for c in range(nchunks):
            deepset(c)
            if c + 2 < nchunks:
                rankA(c + 2)
            if c % 2 == 1:
                edpair((c - 1) // 2)
            if c >= 1:
                mainphase(c - 1)
            if c % 2 == 1 and c >= 3:
                empair((c - 3) // 2)
            if c + 2 < nchunks:
                rankB(c + 2)
            if c + 3 < nchunks:
                xtm_load(c + 3)
            if c + 2 < nchunks:
                # issued after mainphase(c-1): bufs=3 WAR rotation on x_fm
                # is against already-issued readers
                xfm_load(c + 2)
        mainphase(nchunks - 1)
        empair(nchunks // 2 - 1)

    nc.compile()
    return nc


def make_tables(inp):
    """Host-side weight preprocessing -> dict of extra input arrays."""
    f = np.float32
    keys = ("rank_W", "kd_W1", "kd_b1", "kd_W2", "kd_b2", "vd_W1", "vd_b1",
            "vd_W2", "vd_b2", "ed_W1", "ed_b1", "ed_W2", "ed_b2", "km_W1",
            "km_b1", "km_W2", "km_b2", "vm_W1", "vm_b1", "vm_W2", "vm_b2",
            "em_W1", "em_b1", "em_W2", "em_b2")
    g = {k: np.asarray(inp[k], f) for k in keys}

    def A(v):
        return np.ascontiguousarray(v, dtype=f)

    import ml_dtypes

    def Bc(v):
        return np.ascontiguousarray(np.asarray(v, f).astype(ml_dtypes.bfloat16))

    kd_h = np.maximum(g["kd_W1"][:16] + g["kd_b1"][None, :], 0.0)
    Kds16 = kd_h @ g["kd_W2"] + g["kd_b2"][None, :]
    Kds17 = np.vstack([Kds16, np.zeros((1, DIM), f)])
    # km first-layer position rows; extra col 392 = valid-mask generator
    # (one-hot rhs sums to 1 -> relu(col392 row) == mask row)
    KRt17 = np.vstack([g["km_W1"][:16] + g["km_b1"][None, :],
                       np.full((1, 392), -BIG, f)])
    KRt17 = np.hstack([KRt17,
                       np.concatenate([np.ones((16, 1), f),
                                       np.zeros((1, 1), f)]),
                       np.zeros((MAXN1, 1), f)])

    def rep4(tab):
        out = np.zeros((128, tab.shape[1]), f)
        for gi in range(4):
            out[32 * gi:32 * gi + MAXN1] = tab
        return out

    Kds = rep4(Kds17)
    KRt = rep4(KRt17)
    kmW2e = np.vstack([g["km_W2"], g["km_b2"][None, :],
                       np.zeros((1, HID), f)])

    Wz = np.hstack([g["vm_W1"][DIM:2 * DIM], g["km_W1"][MAXN1:MAXN1 + DIM],
                    np.zeros((DIM, 2), f)])
    Wze = np.vstack([Wz, (g["ed_b2"] @ Wz)[None, :]])

    # em first layer; extra col 520 = ones generator via the n one-hot
    EMt = np.hstack([g["em_W1"][HID:HID + MAXN1] + g["em_b1"][None, :],
                     np.ones((MAXN1, 1), f), np.zeros((MAXN1, 1), f)])
    emW1y = np.hstack([g["em_W1"][:HID], np.zeros((HID, 2), f)])
    emW2e = np.vstack([g["em_W2"], g["em_b2"][None, :],
                       np.zeros((1, HID), f)])

    p = np.arange(128)
    iota113 = np.where(p % 32 < MAXN1, p % 32, 99).astype(f)[:, None]
    t = np.arange(NSUB)
    i32g = (t[None, :] // N == np.arange(32)[:, None]).astype(f)
    I32r = np.zeros((128, NSUB), f)
    for gi in range(4):
        I32r[32 * gi:32 * (gi + 1)] = i32g
    S8 = (p[:, None] // 16 == np.arange(8)[None, :]).astype(f)
    LT0 = ((p % 16) < (p // 16)).astype(f)[:, None]
    LT1 = ((p % 16) < (p // 16 + 8)).astype(f)[:, None]

    return {
        "vdW1": Bc(g["vd_W1"]), "b1d": A(g["vd_b1"][:, None]),
        "vdW2": A(g["vd_W2"]), "b2d": A(g["vd_b2"][:, None]),
        "vmW1x": Bc(g["vm_W1"][:DIM]), "b1v": A(g["vm_b1"][:, None]),
        "vmW2": Bc(g["vm_W2"]), "b2v": A(g["vm_b2"][:, None]),
        "Kds": A(Kds), "KRt": A(KRt), "kmW2e": Bc(kmW2e),
        "edW1": A(g["ed_W1"]), "edb1": A(g["ed_b1"][:, None]),
        "edW2": A(g["ed_W2"]), "Wze": A(Wze),
        "emW1y": A(emW1y), "EMt": A(EMt), "emW2e": A(emW2e),
        "rankWb": A(np.tile(g["rank_W"].T, (128, 1))),
        "ident": A(np.eye(128)), "I32r": A(I32r),
        "iota113": A(iota113),
        "iota17": A(np.arange(MAXN1)[:, None]),
        "S8": A(S8), "LT0": A(LT0), "LT1": A(LT1),
        "onesr": A(np.ones((1, 2 * CS))),
        "identF": A(np.eye(128)),
        "iota8_0": A(np.arange(8)[:, None]),
        "iota8_1": A(np.arange(8, 16)[:, None]),
        "iotaTok": A((p % 16).astype(f)[:, None]),
        "iotaQ0": A((p // 16).astype(f)[:, None]),
        "iotaQ1": A((8 + p // 16).astype(f)[:, None]),
    }


_prog_cache = {}


def _get_program(sets_per_core, num_devices):
    key = (sets_per_core, num_devices)
    if key not in _prog_cache:
        _prog_cache[key] = build_program(sets_per_core, num_devices)
    return _prog_cache[key]


def kernel(**inputs):
    nc = _get_program(SC, NCORES)
    tabs = make_tables(inputs)
    x = np.ascontiguousarray(np.asarray(inputs["x"], np.float32))
    n = np.ascontiguousarray(np.asarray(inputs["n"], np.int32))
    in_maps = []
    for c in range(NCORES):
        m = dict(tabs)
        m["x"] = np.ascontiguousarray(
            x[c * SC:(c + 1) * SC].reshape(SC * N, DIM))
        m["n_i"] = n[c * SC:(c + 1) * SC]
        in_maps.append(m)
    res = bass_utils.run_bass_kernel_spmd(nc, in_maps, list(range(NCORES)))
    z = np.concatenate([res.results[c]["z_out"] for c in range(NCORES)], 0)
    return z
